# revision 9
# baseline (speedup 1.0000x reference)
"""2-layer GCN (GCNConv + BatchNorm + ReLU) on 8 trn2 NeuronCores.

Strategy (target-sharded graph parallel):
  - Nodes sharded across 8 cores by target id (12500 targets/core).
  - Edges routed to the core owning their target; self-loops appended as
    ordinary edges; symmetric normalization (deg^-1/2 w deg^-1/2) computed
    on host (pure index/weight preprocessing).
  - Per core, per layer: P = (A_shard @ src)^T accumulated in PSUM via
    one-hot matmuls: for each 128-edge chunk, gather the 128 source rows
    (dma_gather, 4 SWDGE queues), build Sel[e, t] = norm_e * (tgt_e == t)
    on DVE (fused is_equal+mult against an iota tile), and accumulate
    P[:, window] += G_chunk^T @ Sel on the TensorEngine.
  - z = W_l @ P per 512-target block (dense matmul); BN batch stats via
    activation accum_out + cross-core AllReduce; finalize relu(z*s + t);
    PE-transpose back to [node, feat] rows.
  - Between layers: AllGather of y rows so every core can gather any
    source row for layer 2.
  - The bias b is skipped: BatchNorm immediately follows the conv, so a
    constant per-feature shift cancels exactly in (agg - mean).

SPMD: one program for all 8 cores. Per-(block,quadrant,window) chunk
counts are maxed over cores and padded (pad edges: src=0, norm=0).
"""

import math
import numpy as np

from concourse import bass, bacc, mybir, tile
from concourse.bass_utils import run_bass_kernel_spmd
from concourse.masks import make_identity

F32 = mybir.dt.float32
I16 = mybir.dt.int16
AX = mybir.AxisListType
ALU = mybir.AluOpType
ACT = mybir.ActivationFunctionType

EPS = 1e-5


class Cfg:
    def __init__(self, nodes, d, cores, blk, win, qrows, nq=4):
        self.nodes = nodes
        self.d = d
        self.cores = cores
        self.nsh = nodes // cores          # targets per core
        self.blk = blk                     # targets per psum block
        self.win = win                     # targets per Sel window
        self.qrows = qrows                 # rows per gather sub-table (int16 idx)
        self.nquad = math.ceil(nodes / qrows)
        self.nb = math.ceil(self.nsh / blk)
        self.nq = nq                       # swdge queues


FULL = Cfg(nodes=100000, d=128, cores=8, blk=512, win=256, qrows=25000)


def _prep(cfg, x, edge_index, edge_weight):
    """Host preprocessing: normalization, routing, padding.

    Returns (struct, per_core_data):
      struct: dict with nch[b][q][w] chunk counts (shared across cores),
              block chunk offsets, total chunks TC.
      per_core_data: list of dicts with gidx/tgt/nrm arrays.
    """
    N, D = x.shape
    row = edge_index[0].astype(np.int64)
    col = edge_index[1].astype(np.int64)
    sl = np.arange(N, dtype=np.int64)
    rows = np.concatenate([row, sl])
    cols = np.concatenate([col, sl])
    w = np.concatenate([edge_weight.astype(np.float64), np.ones(N)])

    deg = np.bincount(cols, weights=w, minlength=N)
    dinv = np.where(deg > 0, 1.0 / np.sqrt(np.maximum(deg, 1e-30)), 0.0)
    norm = (dinv[rows] * w * dinv[cols]).astype(np.float32)

    nwin_b = []
    for b in range(cfg.nb):
        tgts = min(cfg.blk, cfg.nsh - b * cfg.blk)
        nwin_b.append(math.ceil(tgts / cfg.win))

    # per-core edge buckets
    per_core = []
    counts = np.zeros((cfg.cores, cfg.nb, cfg.nquad, cfg.blk // cfg.win), np.int64)
    for c in range(cfg.cores):
        lo, hi = c * cfg.nsh, (c + 1) * cfg.nsh
        m = (cols >= lo) & (cols < hi)
        r_c = rows[m]
        t_c = cols[m] - lo
        n_c = norm[m]
        b_i = t_c // cfg.blk
        w_i = (t_c % cfg.blk) // cfg.win
        q_i = r_c // cfg.qrows
        key = (b_i * cfg.nquad + q_i) * (cfg.blk // cfg.win) + w_i
        order = np.argsort(key, kind="stable")
        per_core.append(
            dict(r=r_c[order], t=t_c[order], n=n_c[order], key=key[order])
        )
        np.add.at(counts[c], (b_i, q_i, w_i), 1)

    cmax = counts.max(axis=0)  # [nb, nquad, nwin]
    nch = np.ceil(cmax / 128).astype(np.int64)  # chunks per (b,q,w)
    # zero out windows that don't exist in partial blocks
    for b in range(cfg.nb):
        for w in range(cfg.blk // cfg.win):
            if w >= nwin_b[b]:
                assert cmax[b, :, w].max() == 0
                nch[b, :, w] = 0

    TC = int(nch.sum())
    # global chunk offset of each (b,q,w) group, ordered (b, q, w)
    goff = np.zeros((cfg.nb, cfg.nquad, cfg.blk // cfg.win), np.int64)
    acc = 0
    blk_start = []
    for b in range(cfg.nb):
        blk_start.append(acc)
        for q in range(cfg.nquad):
            for w in range(cfg.blk // cfg.win):
                goff[b, q, w] = acc
                acc += nch[b, q, w]
    blk_start.append(acc)
    assert acc == TC

    data = []
    for c in range(cfg.cores):
        pc = per_core[c]
        S = TC * 128
        src = np.zeros(S, np.int64)
        tgt = np.zeros(S, np.float32)
        nrm = np.zeros(S, np.float32)
        pos = 0  # position within pc arrays
        for b in range(cfg.nb):
            for q in range(cfg.nquad):
                for w in range(cfg.blk // cfg.win):
                    k = nch[b, q, w]
                    if k == 0:
                        continue
                    cnt = counts[c, b, q, w]
                    s0 = goff[b, q, w] * 128
                    sl_ = slice(pos, pos + cnt)
                    src[s0 : s0 + cnt] = pc["r"][sl_] - q * cfg.qrows
                    tgt[s0 : s0 + cnt] = (pc["t"][sl_] % cfg.win).astype(np.float32)
                    nrm[s0 : s0 + cnt] = pc["n"][sl_]
                    pos += cnt
        assert pos == len(pc["r"])
        # SBUF layouts
        tgt_sb = tgt.reshape(TC, 128).T.copy()          # [128, TC] f32
        nrm_sb = nrm.reshape(TC, 128).T.copy()          # [128, TC] f32
        # gather idx: per chunk, 128 idxs wrapped into [16, 8] -> 8 int16 cols
        gidx = np.zeros((128, TC * 8), np.int16)
        wrapped = src.reshape(TC * 8, 16).astype(np.int16)  # col j holds idxs 16j..16j+15
        gidx[:16, :] = wrapped.T
        gidx[:] = np.tile(gidx[:16], (8, 1))
        data.append(dict(gidx=gidx, tgt=tgt_sb, nrm=nrm_sb))

    struct = dict(nch=nch, goff=goff, blk_start=blk_start, TC=TC, nwin_b=nwin_b)
    return struct, data


def _build(cfg, struct, nlayers=2):
    """Build + compile the SPMD bass program."""
    nch, goff, blk_start, TC, nwin_b = (
        struct["nch"], struct["goff"], struct["blk_start"], struct["TC"],
        struct["nwin_b"],
    )
    N, D, NB = cfg.nodes, cfg.d, cfg.nb
    NWIN = cfg.blk // cfg.win
    maxblk = max(blk_start[b + 1] - blk_start[b] for b in range(NB))

    nc = bacc.Bacc(
        "TRN2", target_bir_lowering=False, debug=False,
        num_devices=cfg.cores, num_swdge_queues=cfg.nq,
    )
    x_in = nc.declare_dram_parameter("x", [N, D], F32, isOutput=False)
    gidx = nc.declare_dram_parameter("gidx", [128, TC * 8], I16, isOutput=False)
    tgt = nc.declare_dram_parameter("tgt", [128, TC], F32, isOutput=False)
    nrm = nc.declare_dram_parameter("nrm", [128, TC], F32, isOutput=False)
    iot = nc.declare_dram_parameter("iota", [128, cfg.win], F32, isOutput=False)
    wts = nc.declare_dram_parameter("wts", [128, 2 * D], F32, isOutput=False)
    gb = nc.declare_dram_parameter("gb", [128, 4], F32, isOutput=False)
    out_sh = nc.declare_dram_parameter("out", [cfg.nsh, D], F32, isOutput=True)

    rg = [list(range(cfg.cores))]
    qrr = [0]

    def next_q():
        q = qrr[0]
        qrr[0] = (q + 1) % cfg.nq
        return q

    with tile.TileContext(nc) as tc:
        with (
            tc.tile_pool(name="meta", bufs=1) as meta,
            tc.tile_pool(name="gp", bufs=2) as gp,
            tc.tile_pool(name="selp", bufs=4) as selp,
            tc.tile_pool(name="sbp", bufs=2) as sbp,
            tc.tile_pool(name="ytp", bufs=3) as ytp,
            tc.tile_pool(name="stat", bufs=1) as statp,
            tc.tile_pool(name="sm", bufs=1) as smp,
            tc.tile_pool(name="pp", bufs=2, space="PSUM") as pp,
            tc.tile_pool(name="zp", bufs=2, space="PSUM") as zp,
            tc.tile_pool(name="tp", bufs=2, space="PSUM") as tpp,
            tc.tile_pool(name="dr", bufs=1, space="DRAM") as dr,
        ):
            # ---- constants / metadata (loaded once) ----
            idx_sb = meta.tile([128, TC * 8], I16)
            nc.sync.dma_start(out=idx_sb[:], in_=gidx[:])
            tgt_sb = meta.tile([128, TC], F32)
            nc.sync.dma_start(out=tgt_sb[:], in_=tgt[:])
            nrm_sb = meta.tile([128, TC], F32)
            nc.sync.dma_start(out=nrm_sb[:], in_=nrm[:])
            iota_sb = meta.tile([128, cfg.win], F32)
            nc.sync.dma_start(out=iota_sb[:], in_=iot[:])
            wts_sb = meta.tile([128, 2 * D], F32)
            nc.sync.dma_start(out=wts_sb[:], in_=wts[:])
            gb_sb = meta.tile([128, 4], F32)
            nc.sync.dma_start(out=gb_sb[:], in_=gb[:])
            ident = meta.tile([128, 128], F32)
            make_identity(nc, ident[:])

            # ---- DRAM intermediates ----
            y_shard = dr.tile([cfg.nsh, D], F32)
            y_full = dr.tile([N, D], F32, addr_space="Shared")
            z_dram = [dr.tile([128, cfg.nsh], F32, name=f"z_dram{l}") for l in range(2)]
            st_in = [dr.tile([128, 2], F32, name=f"st_in{l}") for l in range(2)]
            st_out = [dr.tile([128, 2], F32, name=f"st_out{l}") for l in range(2)]

            for layer in range(nlayers):
                sums = statp.tile([128, NB], F32, name=f"sums{layer}")
                sumsq = statp.tile([128, NB], F32, name=f"sumsq{layer}")
                for b in range(NB):
                    b0 = blk_start[b]
                    nbc = blk_start[b + 1] - b0
                    nwin = nwin_b[b]
                    bwidth = nwin * cfg.win
                    swidth = min(cfg.blk, cfg.nsh - b * cfg.blk)

                    G = gp.tile([128, maxblk * D], F32, name="G")
                    for q in range(cfg.nquad):
                        ncq = int(nch[b, q, :].sum())
                        if ncq == 0:
                            continue
                        cstart = int(goff[b, q, 0])
                        if layer == 0:
                            src_ap = x_in[q * cfg.qrows : (q + 1) * cfg.qrows, :]
                        else:
                            src_ap = y_full[q * cfg.qrows : (q + 1) * cfg.qrows, :]
                        nc.gpsimd.dma_gather(
                            G[:, (cstart - b0) * D : (cstart - b0 + ncq) * D]
                            .rearrange("p (c d) -> p c d", d=D),
                            src_ap,
                            idx_sb[:, cstart * 8 : (cstart + ncq) * 8],
                            ncq * 128,
                            ncq * 128,
                            D,
                            queue_num=next_q(),
                            single_packet=False,
                        )

                    P = pp.tile([128, cfg.blk], F32, name="P", space="PSUM")
                    for w in range(nwin):
                        cis = []
                        for q in range(cfg.nquad):
                            g0 = int(goff[b, q, w])
                            cis.extend(range(g0, g0 + int(nch[b, q, w])))
                        for k, ci in enumerate(cis):
                            sel = selp.tile([128, cfg.win], F32, name="sel")
                            nc.vector.tensor_scalar(
                                sel[:],
                                iota_sb[:],
                                tgt_sb[:, ci : ci + 1],
                                nrm_sb[:, ci : ci + 1],
                                ALU.is_equal,
                                ALU.mult,
                            )
                            nc.tensor.matmul(
                                P[:, w * cfg.win : (w + 1) * cfg.win],
                                lhsT=G[:, (ci - b0) * D : (ci - b0 + 1) * D],
                                rhs=sel[:],
                                start=(k == 0),
                                stop=(k == len(cis) - 1),
                            )

                    P_sb = sbp.tile([128, cfg.blk], F32, name="P_sb")
                    nc.scalar.copy(P_sb[:, :bwidth], P[:, :bwidth])
                    z_ps = zp.tile([128, cfg.blk], F32, name="z_ps", space="PSUM")
                    nc.tensor.matmul(
                        z_ps[:, :bwidth],
                        lhsT=wts_sb[:, layer * D : (layer + 1) * D],
                        rhs=P_sb[:, :bwidth],
                        start=True,
                        stop=True,
                    )
                    z_sb = sbp.tile([128, cfg.blk], F32, name="z_sb")
                    nc.scalar.activation(
                        z_sb[:, :bwidth], z_ps[:, :bwidth], ACT.Copy,
                        accum_out=sums[:, b : b + 1],
                    )
                    zsq = sbp.tile([128, cfg.blk], F32, name="zsq")
                    nc.scalar.activation(
                        zsq[:, :bwidth], z_ps[:, :bwidth], ACT.Square,
                        accum_out=sumsq[:, b : b + 1],
                    )
                    nc.sync.dma_start(
                        out=z_dram[layer][:, b * cfg.blk : b * cfg.blk + swidth],
                        in_=z_sb[:, :swidth],
                    )

                # ---- global BN stats ----
                ssum = smp.tile([128, 1], F32, name=f"ssum{layer}")
                nc.vector.tensor_reduce(ssum[:], sums[:], AX.X, ALU.add)
                ssq = smp.tile([128, 1], F32, name=f"ssq{layer}")
                nc.vector.tensor_reduce(ssq[:], sumsq[:], AX.X, ALU.add)
                st2 = smp.tile([128, 2], F32, name=f"st2{layer}")
                nc.vector.tensor_copy(st2[:, 0:1], ssum[:])
                nc.vector.tensor_copy(st2[:, 1:2], ssq[:])
                nc.sync.dma_start(out=st_in[layer][:], in_=st2[:])
                nc.gpsimd.collective_compute(
                    "AllReduce", ALU.add, replica_groups=rg,
                    ins=[st_in[layer].opt()], outs=[st_out[layer].opt()],
                )
                st2r = smp.tile([128, 2], F32, name=f"st2r{layer}")
                nc.sync.dma_start(out=st2r[:], in_=st_out[layer][:])
                mean = smp.tile([128, 1], F32, name=f"mean{layer}")
                nc.vector.tensor_scalar_mul(mean[:], st2r[:, 0:1], 1.0 / N)
                ex2 = smp.tile([128, 1], F32, name=f"ex2{layer}")
                nc.vector.tensor_scalar_mul(ex2[:], st2r[:, 1:2], 1.0 / N)
                msq = smp.tile([128, 1], F32, name=f"msq{layer}")
                nc.vector.tensor_tensor(out=msq[:], in0=mean[:], in1=mean[:], op=ALU.mult)
                var = smp.tile([128, 1], F32, name=f"var{layer}")
                nc.vector.tensor_tensor(out=var[:], in0=ex2[:], in1=msq[:], op=ALU.subtract)
                eps_t = smp.tile([128, 1], F32, name=f"eps{layer}")
                nc.vector.memset(eps_t[:], EPS)
                std = smp.tile([128, 1], F32, name=f"std{layer}")
                nc.scalar.activation(std[:], var[:], ACT.Sqrt, bias=eps_t[:])
                rstd = smp.tile([128, 1], F32, name=f"rstd{layer}")
                nc.vector.reciprocal(rstd[:], std[:])
                s_t = smp.tile([128, 1], F32, name=f"s{layer}")
                nc.vector.tensor_tensor(
                    out=s_t[:], in0=gb_sb[:, 2 * layer : 2 * layer + 1], in1=rstd[:],
                    op=ALU.mult,
                )
                ms = smp.tile([128, 1], F32, name=f"ms{layer}")
                nc.vector.tensor_tensor(out=ms[:], in0=mean[:], in1=s_t[:], op=ALU.mult)
                t_t = smp.tile([128, 1], F32, name=f"t{layer}")
                nc.vector.tensor_tensor(
                    out=t_t[:], in0=gb_sb[:, 2 * layer + 1 : 2 * layer + 2], in1=ms[:],
                    op=ALU.subtract,
                )

                # ---- finalize: y = relu(z*s + t), transpose to rows ----
                dest = y_shard if (layer == 0 and nlayers == 2) else out_sh
                for b in range(NB):
                    swidth = min(cfg.blk, cfg.nsh - b * cfg.blk)
                    zin = sbp.tile([128, cfg.blk], F32, name="zin")
                    nc.sync.dma_start(
                        out=zin[:, :swidth],
                        in_=z_dram[layer][:, b * cfg.blk : b * cfg.blk + swidth],
                    )
                    y_sb = sbp.tile([128, cfg.blk], F32, name="y_sb")
                    nc.scalar.activation(
                        y_sb[:, :swidth], zin[:, :swidth], ACT.Relu,
                        bias=t_t[:], scale=s_t[:],
                    )
                    for j in range(math.ceil(swidth / 128)):
                        cw = min(128, swidth - j * 128)
                        tps = tpp.tile([128, 128], F32, name="tps", space="PSUM")
                        nc.tensor.transpose(
                            tps[:cw, :], y_sb[:, j * 128 : j * 128 + cw], ident[:]
                        )
                        yts = ytp.tile([128, 128], F32, name="yts")
                        nc.scalar.copy(yts[:cw, :], tps[:cw, :])
                        r0 = b * cfg.blk + j * 128
                        if layer == 0:
                            nc.sync.dma_start(
                                out=dest[r0 : r0 + cw, :], in_=yts[:cw, :]
                            )
                        else:
                            nc.sync.dma_start(
                                out=dest[r0 : r0 + cw, :], in_=yts[:cw, :]
                            )
                if layer == 0 and nlayers == 2:
                    nc.gpsimd.collective_compute(
                        "AllGather", ALU.bypass, replica_groups=rg,
                        ins=[y_shard.opt()], outs=[y_full.opt()],
                    )

    nc.compile()
    return nc


def _run(cfg, nc, struct, data, x, W, gamma, beta, trace=False):
    N, D = cfg.nodes, cfg.d
    x32 = np.ascontiguousarray(x, dtype=np.float32)
    iota = np.tile(np.arange(cfg.win, dtype=np.float32), (128, 1))
    wts = np.concatenate(
        [np.ascontiguousarray(W[l].T, dtype=np.float32) for l in range(2)], axis=1
    )
    gbarr = np.zeros((128, 4), np.float32)
    for l in range(2):
        gbarr[:, 2 * l] = np.asarray(gamma[l], dtype=np.float32)
        gbarr[:, 2 * l + 1] = np.asarray(beta[l], dtype=np.float32)
    in_maps = []
    for c in range(cfg.cores):
        in_maps.append(
            {
                "x": x32,
                "gidx": data[c]["gidx"],
                "tgt": data[c]["tgt"],
                "nrm": data[c]["nrm"],
                "iota": iota,
                "wts": wts,
                "gb": gbarr,
            }
        )
    res = run_bass_kernel_spmd(
        nc, in_maps, core_ids=list(range(cfg.cores)), trace=trace
    )
    out = np.concatenate([res.results[c]["out"] for c in range(cfg.cores)], axis=0)
    return out, res


def kernel(x, edge_index, edge_weight, W, b, gamma, beta):
    cfg = FULL
    x = np.asarray(x)
    edge_index = np.asarray(edge_index)
    edge_weight = np.asarray(edge_weight)
    W = np.asarray(W)
    gamma = np.asarray(gamma)
    beta = np.asarray(beta)
    struct, data = _prep(cfg, x, edge_index, edge_weight)
    nc = _build(cfg, struct)
    out, _ = _run(cfg, nc, struct, data, x, W, gamma, beta)
    return out.astype(np.float32)


# revision 16
# speedup vs baseline: 1.2763x; 1.2763x over previous
"""2-layer GCN (GCNConv + BatchNorm + ReLU) on 8 trn2 NeuronCores.

Strategy (target-sharded graph parallel):
  - Nodes sharded across 8 cores by target id (12500 targets/core).
  - Edges routed to the core owning their target; self-loops appended as
    ordinary edges; symmetric normalization (deg^-1/2 w deg^-1/2) computed
    on host (pure index/weight preprocessing).
  - Per core, per layer: P = (A_shard @ src)^T accumulated in PSUM via
    one-hot matmuls: for each 128-edge chunk, gather the 128 source rows
    (dma_gather, 4 SWDGE queues), build Sel[e, t] = norm_e * (tgt_e == t)
    on DVE (fused is_equal+mult against an iota tile), and accumulate
    P[:, window] += G_chunk^T @ Sel on the TensorEngine.
  - z = W_l @ P per 512-target block (dense matmul); BN batch stats via
    activation accum_out + cross-core AllReduce; finalize relu(z*s + t);
    PE-transpose back to [node, feat] rows.
  - Between layers: AllGather of y rows so every core can gather any
    source row for layer 2.
  - The bias b is skipped: BatchNorm immediately follows the conv, so a
    constant per-feature shift cancels exactly in (agg - mean).

SPMD: one program for all 8 cores. Per-(block,quadrant,window) chunk
counts are maxed over cores and padded (pad edges: src=0, norm=0).
"""

import math
import numpy as np

from concourse import bass, bacc, mybir, tile
from concourse.bass_utils import run_bass_kernel_spmd
from concourse.masks import make_identity

F32 = mybir.dt.float32
I16 = mybir.dt.int16
AX = mybir.AxisListType
ALU = mybir.AluOpType
ACT = mybir.ActivationFunctionType

EPS = 1e-5


class Cfg:
    def __init__(self, nodes, d, cores, blk, win, qrows, nq=4):
        self.nodes = nodes
        self.d = d
        self.cores = cores
        self.nsh = nodes // cores          # targets per core
        self.blk = blk                     # targets per psum block
        self.win = win                     # targets per Sel window
        self.qrows = qrows                 # rows per gather sub-table (int16 idx)
        self.nquad = math.ceil(nodes / qrows)
        self.nb = math.ceil(self.nsh / blk)
        self.nq = nq                       # swdge queues


FULL = Cfg(nodes=100000, d=128, cores=8, blk=512, win=256, qrows=25000)


def _prep(cfg, x, edge_index, edge_weight):
    """Host preprocessing: normalization, routing, padding.

    Returns (struct, per_core_data):
      struct: dict with nch[b][q][w] chunk counts (shared across cores),
              block chunk offsets, total chunks TC.
      per_core_data: list of dicts with gidx/tgt/nrm arrays.
    """
    N, D = x.shape
    row = edge_index[0].astype(np.int64)
    col = edge_index[1].astype(np.int64)
    sl = np.arange(N, dtype=np.int64)
    rows = np.concatenate([row, sl])
    cols = np.concatenate([col, sl])
    w = np.concatenate([edge_weight.astype(np.float64), np.ones(N)])

    deg = np.bincount(cols, weights=w, minlength=N)
    dinv = np.where(deg > 0, 1.0 / np.sqrt(np.maximum(deg, 1e-30)), 0.0)
    norm = (dinv[rows] * w * dinv[cols]).astype(np.float32)

    # per-core edge buckets: group by (block, quadrant), sorted by target
    per_core = []
    counts = np.zeros((cfg.cores, cfg.nb, cfg.nquad), np.int64)
    for c in range(cfg.cores):
        lo, hi = c * cfg.nsh, (c + 1) * cfg.nsh
        m = (cols >= lo) & (cols < hi)
        r_c = rows[m]
        t_c = cols[m] - lo
        n_c = norm[m]
        b_i = t_c // cfg.blk
        q_i = r_c // cfg.qrows
        key = (b_i * cfg.nquad + q_i) * cfg.nsh + t_c
        order = np.argsort(key, kind="stable")
        per_core.append(dict(r=r_c[order], t=t_c[order], n=n_c[order]))
        np.add.at(counts[c], (b_i, q_i), 1)

    cmax = counts.max(axis=0)  # [nb, nquad]
    nch = np.ceil(cmax / 128).astype(np.int64)  # chunks per (b,q)
    TC = int(nch.sum())
    goff = np.zeros((cfg.nb, cfg.nquad), np.int64)
    acc = 0
    blk_start = []
    for b in range(cfg.nb):
        blk_start.append(acc)
        for q in range(cfg.nquad):
            goff[b, q] = acc
            acc += nch[b, q]
    blk_start.append(acc)
    assert acc == TC

    data = []
    # per-chunk REAL target min/max across cores (to derive psum windows)
    ch_min = np.full(TC, np.iinfo(np.int64).max, np.int64)
    ch_max = np.full(TC, -1, np.int64)
    tgt_all = []
    valid_all = []
    for c in range(cfg.cores):
        pc = per_core[c]
        S = TC * 128
        src = np.zeros(S, np.int64)
        tgt = np.zeros(S, np.int64)       # block-relative target per slot
        valid = np.zeros(S, bool)
        nrm = np.zeros(S, np.float32)
        pos = 0
        for b in range(cfg.nb):
            for q in range(cfg.nquad):
                k = nch[b, q]
                if k == 0:
                    continue
                cnt = counts[c, b, q]
                s0 = goff[b, q] * 128
                sl_ = slice(pos, pos + cnt)
                src[s0 : s0 + cnt] = pc["r"][sl_] - q * cfg.qrows
                tgt[s0 : s0 + cnt] = pc["t"][sl_] - b * cfg.blk
                valid[s0 : s0 + cnt] = True
                nrm[s0 : s0 + cnt] = pc["n"][sl_]
                pos += cnt
        assert pos == len(pc["r"])
        t2 = tgt.reshape(TC, 128)
        v2 = valid.reshape(TC, 128)
        np.minimum(
            ch_min, np.where(v2, t2, np.iinfo(np.int64).max).min(axis=1), out=ch_min
        )
        np.maximum(ch_max, np.where(v2, t2, -1).max(axis=1), out=ch_max)
        tgt_all.append(tgt)
        valid_all.append(valid)
        nrm_sb = nrm.reshape(TC, 128).T.copy()
        gidx = np.zeros((128, TC * 8), np.int16)
        wrapped = src.reshape(TC * 8, 16).astype(np.int16)
        gidx[:16, :] = wrapped.T
        gidx[:] = np.tile(gidx[:16], (8, 1))
        data.append(dict(gidx=gidx, nrm=nrm_sb))

    # windows: per chunk, 32-aligned start covering the real span over all
    # cores. Chunks with no real edges anywhere get [0, 32). The first chunk
    # of every block is forced full-width: its start=True matmul writes (and
    # clears has_written for) the entire psum block.
    empty = ch_max < 0
    ch_min[empty] = 0
    ch_max[empty] = 0
    w0 = (ch_min // 32) * 32
    wid = np.ceil((ch_max + 1 - w0) / 32).astype(np.int64) * 32
    for b in range(cfg.nb):
        c0 = blk_start[b]
        w0[c0] = 0
        wid[c0] = cfg.blk
    wid = np.minimum(wid, cfg.blk - w0)
    assert (wid >= 1).all() and (w0 + wid <= cfg.blk).all()

    w0r = np.repeat(w0, 128)
    for c in range(cfg.cores):
        # pad slots take the chunk's window start (always in range, norm=0)
        tgt = np.where(valid_all[c], tgt_all[c], w0r)
        trel = tgt - w0r
        assert (trel >= 0).all() and (trel < np.repeat(wid, 128)).all()
        data[c]["tgt"] = trel.astype(np.float32).reshape(TC, 128).T.copy()

    struct = dict(
        nch=nch, goff=goff, blk_start=blk_start, TC=TC,
        w0=w0.astype(int), wid=wid.astype(int),
    )
    return struct, data


def _build(cfg, struct, nlayers=2):
    """Build + compile the SPMD bass program."""
    nch, goff, blk_start, TC, w0, wid = (
        struct["nch"], struct["goff"], struct["blk_start"], struct["TC"],
        struct["w0"], struct["wid"],
    )
    N, D, NB = cfg.nodes, cfg.d, cfg.nb
    maxblk = max(blk_start[b + 1] - blk_start[b] for b in range(NB))

    nc = bacc.Bacc(
        "TRN2", target_bir_lowering=False, debug=False,
        num_devices=cfg.cores, num_swdge_queues=cfg.nq,
    )
    x_in = nc.declare_dram_parameter("x", [N, D], F32, isOutput=False)
    gidx = nc.declare_dram_parameter("gidx", [128, TC * 8], I16, isOutput=False)
    tgt = nc.declare_dram_parameter("tgt", [128, TC], F32, isOutput=False)
    nrm = nc.declare_dram_parameter("nrm", [128, TC], F32, isOutput=False)
    iot = nc.declare_dram_parameter("iota", [128, cfg.blk], F32, isOutput=False)
    wts = nc.declare_dram_parameter("wts", [128, 2 * D], F32, isOutput=False)
    gb = nc.declare_dram_parameter("gb", [128, 4], F32, isOutput=False)
    out_sh = nc.declare_dram_parameter("out", [cfg.nsh, D], F32, isOutput=True)

    rg = [list(range(cfg.cores))]
    qrr = [0]

    def next_q():
        q = qrr[0]
        qrr[0] = (q + 1) % cfg.nq
        return q

    with tile.TileContext(nc) as tc:
        with (
            tc.tile_pool(name="meta", bufs=1) as meta,
            tc.tile_pool(name="gp", bufs=2) as gp,
            tc.tile_pool(name="selp", bufs=4) as selp,
            tc.tile_pool(name="sbp", bufs=2) as sbp,
            tc.tile_pool(name="ytp", bufs=3) as ytp,
            tc.tile_pool(name="stat", bufs=1) as statp,
            tc.tile_pool(name="sm", bufs=1) as smp,
            tc.tile_pool(name="pp", bufs=2, space="PSUM") as pp,
            tc.tile_pool(name="zp", bufs=2, space="PSUM") as zp,
            tc.tile_pool(name="tp", bufs=2, space="PSUM") as tpp,
            tc.tile_pool(name="dr", bufs=1, space="DRAM") as dr,
        ):
            # ---- constants / metadata (loaded once) ----
            idx_sb = meta.tile([128, TC * 8], I16)
            nc.sync.dma_start(out=idx_sb[:], in_=gidx[:])
            tgt_sb = meta.tile([128, TC], F32)
            nc.sync.dma_start(out=tgt_sb[:], in_=tgt[:])
            nrm_sb = meta.tile([128, TC], F32)
            nc.sync.dma_start(out=nrm_sb[:], in_=nrm[:])
            iota_sb = meta.tile([128, cfg.blk], F32)
            nc.sync.dma_start(out=iota_sb[:], in_=iot[:])
            wts_sb = meta.tile([128, 2 * D], F32)
            nc.sync.dma_start(out=wts_sb[:], in_=wts[:])
            gb_sb = meta.tile([128, 4], F32)
            nc.sync.dma_start(out=gb_sb[:], in_=gb[:])
            ident = meta.tile([128, 128], F32)
            make_identity(nc, ident[:])

            # ---- DRAM intermediates ----
            y_shard = dr.tile([cfg.nsh, D], F32)
            y_full = dr.tile([N, D], F32, addr_space="Shared")
            z_dram = [dr.tile([128, cfg.nsh], F32, name=f"z_dram{l}") for l in range(2)]
            st_in = [dr.tile([128, 2], F32, name=f"st_in{l}") for l in range(2)]
            st_out = [dr.tile([128, 2], F32, name=f"st_out{l}") for l in range(2)]

            for layer in range(nlayers):
                sums = statp.tile([128, NB], F32, name=f"sums{layer}")
                sumsq = statp.tile([128, NB], F32, name=f"sumsq{layer}")
                for b in range(NB):
                    b0 = blk_start[b]
                    nbc = blk_start[b + 1] - b0
                    swidth = min(cfg.blk, cfg.nsh - b * cfg.blk)

                    G = gp.tile([128, maxblk * D], F32, name="G")
                    for q in range(cfg.nquad):
                        ncq = int(nch[b, q])
                        if ncq == 0:
                            continue
                        cstart = int(goff[b, q])
                        if layer == 0:
                            src_ap = x_in[q * cfg.qrows : (q + 1) * cfg.qrows, :]
                        else:
                            src_ap = y_full[q * cfg.qrows : (q + 1) * cfg.qrows, :]
                        nc.gpsimd.dma_gather(
                            G[:, (cstart - b0) * D : (cstart - b0 + ncq) * D]
                            .rearrange("p (c d) -> p c d", d=D),
                            src_ap,
                            idx_sb[:, cstart * 8 : (cstart + ncq) * 8],
                            ncq * 128,
                            ncq * 128,
                            D,
                            queue_num=next_q(),
                            single_packet=False,
                        )

                    P = pp.tile([128, cfg.blk], F32, name="P", space="PSUM")
                    for k in range(nbc):
                        ci = b0 + k
                        cw0, cwid = int(w0[ci]), int(wid[ci])
                        sel = selp.tile([128, cfg.blk], F32, name="sel")
                        nc.vector.tensor_scalar(
                            sel[:, :cwid],
                            iota_sb[:, :cwid],
                            tgt_sb[:, ci : ci + 1],
                            nrm_sb[:, ci : ci + 1],
                            ALU.is_equal,
                            ALU.mult,
                        )
                        nc.tensor.matmul(
                            P[:, cw0 : cw0 + cwid],
                            lhsT=G[:, k * D : (k + 1) * D],
                            rhs=sel[:, :cwid],
                            start=(k == 0),
                            stop=(k == nbc - 1),
                            skip_group_check=True,
                        )

                    P_sb = sbp.tile([128, cfg.blk], F32, name="P_sb")
                    nc.scalar.copy(P_sb[:], P[:])
                    z_ps = zp.tile([128, cfg.blk], F32, name="z_ps", space="PSUM")
                    nc.tensor.matmul(
                        z_ps[:],
                        lhsT=wts_sb[:, layer * D : (layer + 1) * D],
                        rhs=P_sb[:],
                        start=True,
                        stop=True,
                    )
                    z_sb = sbp.tile([128, cfg.blk], F32, name="z_sb")
                    nc.scalar.activation(
                        z_sb[:], z_ps[:], ACT.Copy,
                        accum_out=sums[:, b : b + 1],
                    )
                    zsq = sbp.tile([128, cfg.blk], F32, name="zsq")
                    nc.scalar.activation(
                        zsq[:], z_ps[:], ACT.Square,
                        accum_out=sumsq[:, b : b + 1],
                    )
                    nc.sync.dma_start(
                        out=z_dram[layer][:, b * cfg.blk : b * cfg.blk + swidth],
                        in_=z_sb[:, :swidth],
                    )

                # ---- global BN stats ----
                ssum = smp.tile([128, 1], F32, name=f"ssum{layer}")
                nc.vector.tensor_reduce(ssum[:], sums[:], AX.X, ALU.add)
                ssq = smp.tile([128, 1], F32, name=f"ssq{layer}")
                nc.vector.tensor_reduce(ssq[:], sumsq[:], AX.X, ALU.add)
                st2 = smp.tile([128, 2], F32, name=f"st2{layer}")
                nc.vector.tensor_copy(st2[:, 0:1], ssum[:])
                nc.vector.tensor_copy(st2[:, 1:2], ssq[:])
                nc.sync.dma_start(out=st_in[layer][:], in_=st2[:])
                nc.gpsimd.collective_compute(
                    "AllReduce", ALU.add, replica_groups=rg,
                    ins=[st_in[layer].opt()], outs=[st_out[layer].opt()],
                )
                st2r = smp.tile([128, 2], F32, name=f"st2r{layer}")
                nc.sync.dma_start(out=st2r[:], in_=st_out[layer][:])
                mean = smp.tile([128, 1], F32, name=f"mean{layer}")
                nc.vector.tensor_scalar_mul(mean[:], st2r[:, 0:1], 1.0 / N)
                ex2 = smp.tile([128, 1], F32, name=f"ex2{layer}")
                nc.vector.tensor_scalar_mul(ex2[:], st2r[:, 1:2], 1.0 / N)
                msq = smp.tile([128, 1], F32, name=f"msq{layer}")
                nc.vector.tensor_tensor(out=msq[:], in0=mean[:], in1=mean[:], op=ALU.mult)
                var = smp.tile([128, 1], F32, name=f"var{layer}")
                nc.vector.tensor_tensor(out=var[:], in0=ex2[:], in1=msq[:], op=ALU.subtract)
                eps_t = smp.tile([128, 1], F32, name=f"eps{layer}")
                nc.vector.memset(eps_t[:], EPS)
                std = smp.tile([128, 1], F32, name=f"std{layer}")
                nc.scalar.activation(std[:], var[:], ACT.Sqrt, bias=eps_t[:])
                rstd = smp.tile([128, 1], F32, name=f"rstd{layer}")
                nc.vector.reciprocal(rstd[:], std[:])
                s_t = smp.tile([128, 1], F32, name=f"s{layer}")
                nc.vector.tensor_tensor(
                    out=s_t[:], in0=gb_sb[:, 2 * layer : 2 * layer + 1], in1=rstd[:],
                    op=ALU.mult,
                )
                ms = smp.tile([128, 1], F32, name=f"ms{layer}")
                nc.vector.tensor_tensor(out=ms[:], in0=mean[:], in1=s_t[:], op=ALU.mult)
                t_t = smp.tile([128, 1], F32, name=f"t{layer}")
                nc.vector.tensor_tensor(
                    out=t_t[:], in0=gb_sb[:, 2 * layer + 1 : 2 * layer + 2], in1=ms[:],
                    op=ALU.subtract,
                )

                # ---- finalize: y = relu(z*s + t), transpose to rows ----
                dest = y_shard if (layer == 0 and nlayers == 2) else out_sh
                for b in range(NB):
                    swidth = min(cfg.blk, cfg.nsh - b * cfg.blk)
                    zin = sbp.tile([128, cfg.blk], F32, name="zin")
                    nc.sync.dma_start(
                        out=zin[:, :swidth],
                        in_=z_dram[layer][:, b * cfg.blk : b * cfg.blk + swidth],
                    )
                    y_sb = sbp.tile([128, cfg.blk], F32, name="y_sb")
                    nc.scalar.activation(
                        y_sb[:, :swidth], zin[:, :swidth], ACT.Relu,
                        bias=t_t[:], scale=s_t[:],
                    )
                    for j in range(math.ceil(swidth / 128)):
                        cw = min(128, swidth - j * 128)
                        tps = tpp.tile([128, 128], F32, name="tps", space="PSUM")
                        nc.tensor.transpose(
                            tps[:cw, :], y_sb[:, j * 128 : j * 128 + cw], ident[:]
                        )
                        yts = ytp.tile([128, 128], F32, name="yts")
                        nc.scalar.copy(yts[:cw, :], tps[:cw, :])
                        r0 = b * cfg.blk + j * 128
                        if layer == 0:
                            nc.sync.dma_start(
                                out=dest[r0 : r0 + cw, :], in_=yts[:cw, :]
                            )
                        else:
                            nc.sync.dma_start(
                                out=dest[r0 : r0 + cw, :], in_=yts[:cw, :]
                            )
                if layer == 0 and nlayers == 2:
                    nc.gpsimd.collective_compute(
                        "AllGather", ALU.bypass, replica_groups=rg,
                        ins=[y_shard.opt()], outs=[y_full.opt()],
                    )

    nc.compile()
    return nc


def _run(cfg, nc, struct, data, x, W, gamma, beta, trace=False):
    N, D = cfg.nodes, cfg.d
    x32 = np.ascontiguousarray(x, dtype=np.float32)
    iota = np.tile(np.arange(cfg.blk, dtype=np.float32), (128, 1))
    wts = np.concatenate(
        [np.ascontiguousarray(W[l].T, dtype=np.float32) for l in range(2)], axis=1
    )
    gbarr = np.zeros((128, 4), np.float32)
    for l in range(2):
        gbarr[:, 2 * l] = np.asarray(gamma[l], dtype=np.float32)
        gbarr[:, 2 * l + 1] = np.asarray(beta[l], dtype=np.float32)
    in_maps = []
    for c in range(cfg.cores):
        in_maps.append(
            {
                "x": x32,
                "gidx": data[c]["gidx"],
                "tgt": data[c]["tgt"],
                "nrm": data[c]["nrm"],
                "iota": iota,
                "wts": wts,
                "gb": gbarr,
            }
        )
    res = run_bass_kernel_spmd(
        nc, in_maps, core_ids=list(range(cfg.cores)), trace=trace
    )
    out = np.concatenate([res.results[c]["out"] for c in range(cfg.cores)], axis=0)
    return out, res


def kernel(x, edge_index, edge_weight, W, b, gamma, beta):
    cfg = FULL
    x = np.asarray(x)
    edge_index = np.asarray(edge_index)
    edge_weight = np.asarray(edge_weight)
    W = np.asarray(W)
    gamma = np.asarray(gamma)
    beta = np.asarray(beta)
    struct, data = _prep(cfg, x, edge_index, edge_weight)
    nc = _build(cfg, struct)
    out, _ = _run(cfg, nc, struct, data, x, W, gamma, beta)
    return out.astype(np.float32)


# revision 21
# speedup vs baseline: 1.3044x; 1.0220x over previous
"""2-layer GCN (GCNConv + BatchNorm + ReLU) on 8 trn2 NeuronCores.

Strategy (target-sharded graph parallel):
  - Nodes sharded across 8 cores by target id (12500 targets/core).
  - Edges routed to the core owning their target; self-loops appended as
    ordinary edges; symmetric normalization (deg^-1/2 w deg^-1/2) computed
    on host (pure index/weight preprocessing).
  - Per core, per layer: P = (A_shard @ src)^T accumulated in PSUM via
    one-hot matmuls: for each 128-edge chunk, gather the 128 source rows
    (dma_gather, 4 SWDGE queues), build Sel[e, t] = norm_e * (tgt_e == t)
    on DVE (fused is_equal+mult against an iota tile), and accumulate
    P[:, window] += G_chunk^T @ Sel on the TensorEngine.
  - z = W_l @ P per 512-target block (dense matmul); BN batch stats via
    activation accum_out + cross-core AllReduce; finalize relu(z*s + t);
    PE-transpose back to [node, feat] rows.
  - Between layers: AllGather of y rows so every core can gather any
    source row for layer 2.
  - The bias b is skipped: BatchNorm immediately follows the conv, so a
    constant per-feature shift cancels exactly in (agg - mean).

SPMD: one program for all 8 cores. Per-(block,quadrant,window) chunk
counts are maxed over cores and padded (pad edges: src=0, norm=0).
"""

import math
import numpy as np

from concourse import bass, bacc, mybir, tile
from concourse.bass_utils import run_bass_kernel_spmd
from concourse.masks import make_identity

F32 = mybir.dt.float32
I16 = mybir.dt.int16
AX = mybir.AxisListType
ALU = mybir.AluOpType
ACT = mybir.ActivationFunctionType

EPS = 1e-5


class Cfg:
    def __init__(self, nodes, d, cores, blk, win, qrows, nq=4):
        self.nodes = nodes
        self.d = d
        self.cores = cores
        self.nsh = nodes // cores          # targets per core
        self.blk = blk                     # targets per psum block
        self.win = win                     # targets per Sel window
        self.qrows = qrows                 # rows per gather sub-table (int16 idx)
        self.nquad = math.ceil(nodes / qrows)
        self.nb = math.ceil(self.nsh / blk)
        self.nq = nq                       # swdge queues


FULL = Cfg(nodes=100000, d=128, cores=8, blk=512, win=256, qrows=25000)


def _prep(cfg, x, edge_index, edge_weight):
    """Host preprocessing: normalization, routing, padding.

    Returns (struct, per_core_data):
      struct: dict with nch[b][q][w] chunk counts (shared across cores),
              block chunk offsets, total chunks TC.
      per_core_data: list of dicts with gidx/tgt/nrm arrays.
    """
    N, D = x.shape
    row = edge_index[0].astype(np.int64)
    col = edge_index[1].astype(np.int64)
    sl = np.arange(N, dtype=np.int64)
    rows = np.concatenate([row, sl])
    cols = np.concatenate([col, sl])
    w = np.concatenate([edge_weight.astype(np.float64), np.ones(N)])

    deg = np.bincount(cols, weights=w, minlength=N)
    dinv = np.where(deg > 0, 1.0 / np.sqrt(np.maximum(deg, 1e-30)), 0.0)
    norm = (dinv[rows] * w * dinv[cols]).astype(np.float32)

    # per-core edge buckets: group by (block, quadrant), sorted by target
    per_core = []
    counts = np.zeros((cfg.cores, cfg.nb, cfg.nquad), np.int64)
    for c in range(cfg.cores):
        lo, hi = c * cfg.nsh, (c + 1) * cfg.nsh
        m = (cols >= lo) & (cols < hi)
        r_c = rows[m]
        t_c = cols[m] - lo
        n_c = norm[m]
        b_i = t_c // cfg.blk
        q_i = r_c // cfg.qrows
        key = (b_i * cfg.nquad + q_i) * cfg.nsh + t_c
        order = np.argsort(key, kind="stable")
        per_core.append(dict(r=r_c[order], t=t_c[order], n=n_c[order]))
        np.add.at(counts[c], (b_i, q_i), 1)

    cmax = counts.max(axis=0)  # [nb, nquad]
    c16 = np.ceil(cmax / 16).astype(np.int64) * 16   # gather rows per (b,q)
    nch = np.ceil(cmax / 128).astype(np.int64)  # chunks per (b,q)
    TC = int(nch.sum())
    goff = np.zeros((cfg.nb, cfg.nquad), np.int64)
    acc = 0
    blk_start = []
    for b in range(cfg.nb):
        blk_start.append(acc)
        for q in range(cfg.nquad):
            goff[b, q] = acc
            acc += nch[b, q]
    blk_start.append(acc)
    assert acc == TC

    data = []
    # per-chunk REAL target min/max across cores (to derive psum windows)
    ch_min = np.full(TC, np.iinfo(np.int64).max, np.int64)
    ch_max = np.full(TC, -1, np.int64)
    tgt_all = []
    valid_all = []
    for c in range(cfg.cores):
        pc = per_core[c]
        S = TC * 128
        src = np.zeros(S, np.int64)
        tgt = np.zeros(S, np.int64)       # block-relative target per slot
        valid = np.zeros(S, bool)
        nrm = np.zeros(S, np.float32)
        pos = 0
        for b in range(cfg.nb):
            for q in range(cfg.nquad):
                k = nch[b, q]
                if k == 0:
                    continue
                cnt = counts[c, b, q]
                s0 = goff[b, q] * 128
                sl_ = slice(pos, pos + cnt)
                src[s0 : s0 + cnt] = pc["r"][sl_] - q * cfg.qrows
                tgt[s0 : s0 + cnt] = pc["t"][sl_] - b * cfg.blk
                valid[s0 : s0 + cnt] = True
                nrm[s0 : s0 + cnt] = pc["n"][sl_]
                pos += cnt
        assert pos == len(pc["r"])
        t2 = tgt.reshape(TC, 128)
        v2 = valid.reshape(TC, 128)
        np.minimum(
            ch_min, np.where(v2, t2, np.iinfo(np.int64).max).min(axis=1), out=ch_min
        )
        np.maximum(ch_max, np.where(v2, t2, -1).max(axis=1), out=ch_max)
        tgt_all.append(tgt)
        valid_all.append(valid)
        nrm_sb = nrm.reshape(TC, 128).T.copy()
        gidx = np.zeros((128, TC * 8), np.int16)
        wrapped = src.reshape(TC * 8, 16).astype(np.int16)
        gidx[:16, :] = wrapped.T
        gidx[:] = np.tile(gidx[:16], (8, 1))
        data.append(dict(gidx=gidx, nrm=nrm_sb))

    # windows: per chunk, 32-aligned start covering the real span over all
    # cores. Chunks with no real edges anywhere get [0, 32). The first chunk
    # of every block is forced full-width: its start=True matmul writes (and
    # clears has_written for) the entire psum block.
    empty = ch_max < 0
    ch_min[empty] = 0
    ch_max[empty] = 0
    w0 = (ch_min // 32) * 32
    wid = np.ceil((ch_max + 1 - w0) / 32).astype(np.int64) * 32
    for b in range(cfg.nb):
        c0 = blk_start[b]
        w0[c0] = 0
        wid[c0] = cfg.blk
    wid = np.minimum(wid, cfg.blk - w0)
    assert (wid >= 1).all() and (w0 + wid <= cfg.blk).all()

    w0r = np.repeat(w0, 128)
    for c in range(cfg.cores):
        # pad slots take the chunk's window start (always in range, norm=0)
        tgt = np.where(valid_all[c], tgt_all[c], w0r)
        trel = tgt - w0r
        assert (trel >= 0).all() and (trel < np.repeat(wid, 128)).all()
        data[c]["tgt"] = trel.astype(np.float32).reshape(TC, 128).T.copy()

    struct = dict(
        nch=nch, goff=goff, blk_start=blk_start, TC=TC,
        w0=w0.astype(int), wid=wid.astype(int), c16=c16,
    )
    return struct, data


def _build(cfg, struct, nlayers=2):
    """Build + compile the SPMD bass program."""
    nch, goff, blk_start, TC, w0, wid = (
        struct["nch"], struct["goff"], struct["blk_start"], struct["TC"],
        struct["w0"], struct["wid"],
    )
    N, D, NB = cfg.nodes, cfg.d, cfg.nb
    maxblk = max(blk_start[b + 1] - blk_start[b] for b in range(NB))

    nc = bacc.Bacc(
        "TRN2", target_bir_lowering=False, debug=False,
        num_devices=cfg.cores, num_swdge_queues=cfg.nq,
    )
    x_in = nc.declare_dram_parameter("x", [N, D], F32, isOutput=False)
    gidx = nc.declare_dram_parameter("gidx", [128, TC * 8], I16, isOutput=False)
    tgt = nc.declare_dram_parameter("tgt", [128, TC], F32, isOutput=False)
    nrm = nc.declare_dram_parameter("nrm", [128, TC], F32, isOutput=False)
    iot = nc.declare_dram_parameter("iota", [128, cfg.blk], F32, isOutput=False)
    wts = nc.declare_dram_parameter("wts", [128, 2 * D], F32, isOutput=False)
    gb = nc.declare_dram_parameter("gb", [128, 4], F32, isOutput=False)
    out_sh = nc.declare_dram_parameter("out", [cfg.nsh, D], F32, isOutput=True)

    rg = [list(range(cfg.cores))]
    qrr = [0]

    def next_q():
        q = qrr[0]
        qrr[0] = (q + 1) % cfg.nq
        return q

    with tile.TileContext(nc) as tc:
        with (
            tc.tile_pool(name="meta", bufs=1) as meta,
            tc.tile_pool(name="gp", bufs=2) as gp,
            tc.tile_pool(name="selp", bufs=8) as selp,
            tc.tile_pool(name="sbp", bufs=2) as sbp,
            tc.tile_pool(name="ytp", bufs=3) as ytp,
            tc.tile_pool(name="stat", bufs=1) as statp,
            tc.tile_pool(name="sm", bufs=1) as smp,
            tc.tile_pool(name="pp", bufs=2, space="PSUM") as pp,
            tc.tile_pool(name="zp", bufs=2, space="PSUM") as zp,
            tc.tile_pool(name="tp", bufs=2, space="PSUM") as tpp,
            tc.tile_pool(name="dr", bufs=1, space="DRAM") as dr,
        ):
            # ---- constants / metadata (loaded once) ----
            idx_sb = meta.tile([128, TC * 8], I16)
            nc.sync.dma_start(out=idx_sb[:], in_=gidx[:])
            tgt_sb = meta.tile([128, TC], F32)
            nc.sync.dma_start(out=tgt_sb[:], in_=tgt[:])
            nrm_sb = meta.tile([128, TC], F32)
            nc.sync.dma_start(out=nrm_sb[:], in_=nrm[:])
            iota_sb = meta.tile([128, cfg.blk], F32)
            nc.sync.dma_start(out=iota_sb[:], in_=iot[:])
            wts_sb = meta.tile([128, 2 * D], F32)
            nc.sync.dma_start(out=wts_sb[:], in_=wts[:])
            gb_sb = meta.tile([128, 4], F32)
            nc.sync.dma_start(out=gb_sb[:], in_=gb[:])
            ident = meta.tile([128, 128], F32)
            make_identity(nc, ident[:])
            # pre-zero both gather buffers: trailing slots of the last chunk
            # of a (b,q) group are never written by dma_gather (num_idxs is
            # exact), and the very first use must not read NaN garbage.
            for _ in range(2):
                gz = gp.tile([128, maxblk * D], F32, name="G")
                nc.vector.memset(gz[:], 0.0)

            # ---- DRAM intermediates ----
            y_shard = dr.tile([cfg.nsh, D], F32)
            y_full = dr.tile([N, D], F32, addr_space="Shared")
            z_dram = [dr.tile([128, cfg.nsh], F32, name=f"z_dram{l}") for l in range(2)]
            st_in = [dr.tile([128, 2], F32, name=f"st_in{l}") for l in range(2)]
            st_out = [dr.tile([128, 2], F32, name=f"st_out{l}") for l in range(2)]

            for layer in range(nlayers):
                sums = statp.tile([128, NB], F32, name=f"sums{layer}")
                sumsq = statp.tile([128, NB], F32, name=f"sumsq{layer}")
                for b in range(NB):
                    b0 = blk_start[b]
                    nbc = blk_start[b + 1] - b0
                    swidth = min(cfg.blk, cfg.nsh - b * cfg.blk)

                    G = gp.tile([128, maxblk * D], F32, name="G")
                    for q in range(cfg.nquad):
                        ncq = int(nch[b, q])
                        if ncq == 0:
                            continue
                        cstart = int(goff[b, q])
                        if layer == 0:
                            src_ap = x_in[q * cfg.qrows : (q + 1) * cfg.qrows, :]
                        else:
                            src_ap = y_full[q * cfg.qrows : (q + 1) * cfg.qrows, :]
                        nrows = int(struct["c16"][b, q])
                        nc.gpsimd.dma_gather(
                            G[:, (cstart - b0) * D : (cstart - b0 + ncq) * D]
                            .rearrange("p (c d) -> p c d", d=D),
                            src_ap,
                            idx_sb[:, cstart * 8 : (cstart + ncq) * 8],
                            nrows,
                            nrows,
                            D,
                            queue_num=next_q(),
                            single_packet=False,
                        )

                    P = pp.tile([128, cfg.blk], F32, name="P", space="PSUM")
                    for k in range(nbc):
                        ci = b0 + k
                        cw0, cwid = int(w0[ci]), int(wid[ci])
                        sel = selp.tile([128, cfg.blk], F32, name="sel")
                        nc.vector.tensor_scalar(
                            sel[:, :cwid],
                            iota_sb[:, :cwid],
                            tgt_sb[:, ci : ci + 1],
                            nrm_sb[:, ci : ci + 1],
                            ALU.is_equal,
                            ALU.mult,
                        )
                        nc.tensor.matmul(
                            P[:, cw0 : cw0 + cwid],
                            lhsT=G[:, k * D : (k + 1) * D],
                            rhs=sel[:, :cwid],
                            start=(k == 0),
                            stop=(k == nbc - 1),
                            skip_group_check=True,
                        )

                    P_sb = sbp.tile([128, cfg.blk], F32, name="P_sb")
                    nc.scalar.copy(P_sb[:], P[:])
                    z_ps = zp.tile([128, cfg.blk], F32, name="z_ps", space="PSUM")
                    nc.tensor.matmul(
                        z_ps[:],
                        lhsT=wts_sb[:, layer * D : (layer + 1) * D],
                        rhs=P_sb[:],
                        start=True,
                        stop=True,
                    )
                    z_sb = sbp.tile([128, cfg.blk], F32, name="z_sb")
                    nc.scalar.activation(
                        z_sb[:], z_ps[:], ACT.Copy,
                        accum_out=sums[:, b : b + 1],
                    )
                    zsq = sbp.tile([128, cfg.blk], F32, name="zsq")
                    nc.scalar.activation(
                        zsq[:], z_ps[:], ACT.Square,
                        accum_out=sumsq[:, b : b + 1],
                    )
                    nc.sync.dma_start(
                        out=z_dram[layer][:, b * cfg.blk : b * cfg.blk + swidth],
                        in_=z_sb[:, :swidth],
                    )

                # ---- global BN stats ----
                ssum = smp.tile([128, 1], F32, name=f"ssum{layer}")
                nc.vector.tensor_reduce(ssum[:], sums[:], AX.X, ALU.add)
                ssq = smp.tile([128, 1], F32, name=f"ssq{layer}")
                nc.vector.tensor_reduce(ssq[:], sumsq[:], AX.X, ALU.add)
                st2 = smp.tile([128, 2], F32, name=f"st2{layer}")
                nc.vector.tensor_copy(st2[:, 0:1], ssum[:])
                nc.vector.tensor_copy(st2[:, 1:2], ssq[:])
                nc.sync.dma_start(out=st_in[layer][:], in_=st2[:])
                nc.gpsimd.collective_compute(
                    "AllReduce", ALU.add, replica_groups=rg,
                    ins=[st_in[layer].opt()], outs=[st_out[layer].opt()],
                )
                st2r = smp.tile([128, 2], F32, name=f"st2r{layer}")
                nc.sync.dma_start(out=st2r[:], in_=st_out[layer][:])
                mean = smp.tile([128, 1], F32, name=f"mean{layer}")
                nc.vector.tensor_scalar_mul(mean[:], st2r[:, 0:1], 1.0 / N)
                ex2 = smp.tile([128, 1], F32, name=f"ex2{layer}")
                nc.vector.tensor_scalar_mul(ex2[:], st2r[:, 1:2], 1.0 / N)
                msq = smp.tile([128, 1], F32, name=f"msq{layer}")
                nc.vector.tensor_tensor(out=msq[:], in0=mean[:], in1=mean[:], op=ALU.mult)
                var = smp.tile([128, 1], F32, name=f"var{layer}")
                nc.vector.tensor_tensor(out=var[:], in0=ex2[:], in1=msq[:], op=ALU.subtract)
                eps_t = smp.tile([128, 1], F32, name=f"eps{layer}")
                nc.vector.memset(eps_t[:], EPS)
                std = smp.tile([128, 1], F32, name=f"std{layer}")
                nc.scalar.activation(std[:], var[:], ACT.Sqrt, bias=eps_t[:])
                rstd = smp.tile([128, 1], F32, name=f"rstd{layer}")
                nc.vector.reciprocal(rstd[:], std[:])
                s_t = smp.tile([128, 1], F32, name=f"s{layer}")
                nc.vector.tensor_tensor(
                    out=s_t[:], in0=gb_sb[:, 2 * layer : 2 * layer + 1], in1=rstd[:],
                    op=ALU.mult,
                )
                ms = smp.tile([128, 1], F32, name=f"ms{layer}")
                nc.vector.tensor_tensor(out=ms[:], in0=mean[:], in1=s_t[:], op=ALU.mult)
                t_t = smp.tile([128, 1], F32, name=f"t{layer}")
                nc.vector.tensor_tensor(
                    out=t_t[:], in0=gb_sb[:, 2 * layer + 1 : 2 * layer + 2], in1=ms[:],
                    op=ALU.subtract,
                )

                # ---- finalize: y = relu(z*s + t), transpose to rows ----
                dest = y_shard if (layer == 0 and nlayers == 2) else out_sh
                for b in range(NB):
                    swidth = min(cfg.blk, cfg.nsh - b * cfg.blk)
                    zin = sbp.tile([128, cfg.blk], F32, name="zin")
                    nc.sync.dma_start(
                        out=zin[:, :swidth],
                        in_=z_dram[layer][:, b * cfg.blk : b * cfg.blk + swidth],
                    )
                    y_sb = sbp.tile([128, cfg.blk], F32, name="y_sb")
                    nc.scalar.activation(
                        y_sb[:, :swidth], zin[:, :swidth], ACT.Relu,
                        bias=t_t[:], scale=s_t[:],
                    )
                    for j in range(math.ceil(swidth / 128)):
                        cw = min(128, swidth - j * 128)
                        tps = tpp.tile([128, 128], F32, name="tps", space="PSUM")
                        nc.tensor.transpose(
                            tps[:cw, :], y_sb[:, j * 128 : j * 128 + cw], ident[:]
                        )
                        yts = ytp.tile([128, 128], F32, name="yts")
                        nc.scalar.copy(yts[:cw, :], tps[:cw, :])
                        r0 = b * cfg.blk + j * 128
                        if layer == 0:
                            nc.sync.dma_start(
                                out=dest[r0 : r0 + cw, :], in_=yts[:cw, :]
                            )
                        else:
                            nc.sync.dma_start(
                                out=dest[r0 : r0 + cw, :], in_=yts[:cw, :]
                            )
                if layer == 0 and nlayers == 2:
                    nc.gpsimd.collective_compute(
                        "AllGather", ALU.bypass, replica_groups=rg,
                        ins=[y_shard.opt()], outs=[y_full.opt()],
                    )

    nc.compile()
    return nc


def _run(cfg, nc, struct, data, x, W, gamma, beta, trace=False):
    N, D = cfg.nodes, cfg.d
    x32 = np.ascontiguousarray(x, dtype=np.float32)
    iota = np.tile(np.arange(cfg.blk, dtype=np.float32), (128, 1))
    wts = np.concatenate(
        [np.ascontiguousarray(W[l].T, dtype=np.float32) for l in range(2)], axis=1
    )
    gbarr = np.zeros((128, 4), np.float32)
    for l in range(2):
        gbarr[:, 2 * l] = np.asarray(gamma[l], dtype=np.float32)
        gbarr[:, 2 * l + 1] = np.asarray(beta[l], dtype=np.float32)
    in_maps = []
    for c in range(cfg.cores):
        in_maps.append(
            {
                "x": x32,
                "gidx": data[c]["gidx"],
                "tgt": data[c]["tgt"],
                "nrm": data[c]["nrm"],
                "iota": iota,
                "wts": wts,
                "gb": gbarr,
            }
        )
    res = run_bass_kernel_spmd(
        nc, in_maps, core_ids=list(range(cfg.cores)), trace=trace
    )
    out = np.concatenate([res.results[c]["out"] for c in range(cfg.cores)], axis=0)
    return out, res


def kernel(x, edge_index, edge_weight, W, b, gamma, beta):
    cfg = FULL
    x = np.asarray(x)
    edge_index = np.asarray(edge_index)
    edge_weight = np.asarray(edge_weight)
    W = np.asarray(W)
    gamma = np.asarray(gamma)
    beta = np.asarray(beta)
    struct, data = _prep(cfg, x, edge_index, edge_weight)
    nc = _build(cfg, struct)
    out, _ = _run(cfg, nc, struct, data, x, W, gamma, beta)
    return out.astype(np.float32)
